# revision 6
# baseline (speedup 1.0000x reference)
"""Mixtral sparse MoE block on 8 Trainium2 NeuronCores (expert parallelism).

Contract: kernel(**inputs) takes the FULL unsharded inputs and returns the
FULL output (out[B,S,H], router_logits[T,E]) matching reference().

Strategy
--------
Expert parallelism: core c owns experts 4c..4c+3.  The routing metadata
(which token goes to which expert slot) depends only on `selected_experts`,
which is host-visible, so the host performs the dispatch (gather tokens into
per-expert capacity buffers) and the combine (gather-add of weighted expert
outputs) as the shard/unshard steps.  Everything data-dependent runs on
device:

  - router logits per slot (matmul against the replicated gate),
  - per-slot combine weight w = sigmoid(l_own - l_other)   (this equals the
    reference's renormalized top-k softmax weight exactly - the full-E
    partition function cancels),
  - the three expert GEMMs with fused SwiGLU, in float32r (full-rate PE,
    fp32 storage so HBM traffic is unchanged),
  - scaling of the expert output rows by the combine weight.

Weights stream HBM->SBUF->PE exactly once (~138 MB/core), which is the
memory roofline this problem is graded against.
"""

import math
import numpy as np

import concourse.bass as bass
import concourse.mybir as mybir
import concourse.tile as tile
from concourse import bacc
from concourse.bass_utils import run_bass_kernel_spmd

F32 = mybir.dt.float32
F32R = mybir.dt.float32r
AF = mybir.ActivationFunctionType
ALU = mybir.AluOpType

# Problem shape (hardcoded per spec).
B, S, H, F, E, K = 2, 1024, 1024, 2816, 32, 2
T = B * S
CREF = 512            # reference per-expert capacity
NC = 8                # cores
EPC = E // NC         # experts per core = 4
HK = H // 128         # 8 contraction tiles for H
FT = F // 128         # 22 tiles for F
FP = FT // 2          # 11 tile-pairs (1MB DMA granularity)


# ----------------------------------------------------------------------------
# Bass kernel builder (per-core program, identical on all 8 cores)
# ----------------------------------------------------------------------------

def build_nc(Cp: int, repeat: int = 1):
    """Build the per-core Bass program for capacity Cp (multiple of 128).

    repeat > 1 wraps the whole body in a hardware loop (used only for
    timing measurements; outputs are rewritten identically each iteration).
    """
    CPT = Cp // 128           # capacity tiles per expert
    NST = EPC * CPT           # slot tiles per core

    nc = bacc.Bacc("TRN2", target_bir_lowering=False, debug=False,
                   num_devices=NC)

    wb1 = nc.dram_tensor("wb1", [EPC, FP, 128, 2048], F32R, kind="ExternalInput").ap()
    wb3 = nc.dram_tensor("wb3", [EPC, FP, 128, 2048], F32R, kind="ExternalInput").ap()
    wb2 = nc.dram_tensor("wb2", [EPC, FP, 128, 2048], F32R, kind="ExternalInput").ap()
    xg = nc.dram_tensor("xg", [EPC, 128, HK * Cp], F32R, kind="ExternalInput").ap()
    gwt = nc.dram_tensor("gwt", [128, HK * 32], F32R, kind="ExternalInput").ap()
    ohd = nc.dram_tensor("ohd", [128, NST * 32], F32, kind="ExternalInput").ap()
    Y = nc.dram_tensor("Y", [EPC, CPT, 2, 128, 512], F32, kind="ExternalOutput").ap()
    LS = nc.dram_tensor("LS", [NST, 128, 32], F32, kind="ExternalOutput").ap()

    with tile.TileContext(nc) as tc:
        import contextlib
        ctx = contextlib.ExitStack()
        with ctx:
            xg_pool = ctx.enter_context(tc.tile_pool(name="xg", bufs=EPC + 1))
            wt_pool = ctx.enter_context(tc.tile_pool(name="wt", bufs=3))
            ht_pool = ctx.enter_context(tc.tile_pool(name="ht", bufs=FT + 2))
            sm_pool = ctx.enter_context(tc.tile_pool(name="sm", bufs=3))
            yw_pool = ctx.enter_context(tc.tile_pool(name="yw", bufs=4))
            cst_pool = ctx.enter_context(tc.tile_pool(name="cst", bufs=1))
            psA = ctx.enter_context(tc.tile_pool(name="psA", bufs=4, space="PSUM"))
            psB = ctx.enter_context(tc.tile_pool(name="psB", bufs=4, space="PSUM"))

            def body():
                # --- constants / activations resident in SBUF ---
                gwt_t = cst_pool.tile([128, HK * 32], F32R, tag="gwt", name="gwt_t")
                nc.sync.dma_start(out=gwt_t[:], in_=gwt[:])
                ohd_t = cst_pool.tile([128, NST * 32], F32, tag="ohd", name="ohd_t")
                nc.sync.dma_start(out=ohd_t[:], in_=ohd[:])
                wcol = cst_pool.tile([128, NST], F32, tag="wcol", name="wcol")

                xg_t = []
                for e in range(EPC):
                    t = xg_pool.tile([128, HK * Cp], F32R, tag="xg", name=f"xg_{e}")
                    nc.sync.dma_start(out=t[:], in_=xg[e])
                    xg_t.append(t)

                # --- router: slot logits, d = l_own - l_other, w = sigmoid(d)
                for e in range(EPC):
                    for cpt in range(CPT):
                        st = e * CPT + cpt
                        pl_full = psA.tile([128, 512], F32, tag="y", name=f"pl_{st}")
                        pl = pl_full[:, :32]
                        for hk in range(HK):
                            nc.tensor.matmul(
                                pl,
                                xg_t[e][:, hk * Cp + cpt * 128:hk * Cp + cpt * 128 + 128],
                                gwt_t[:, hk * 32:(hk + 1) * 32],
                                start=(hk == 0), stop=(hk == HK - 1),
                            )
                        lt = sm_pool.tile([128, 32], F32, tag="lt", name=f"lt_{st}")
                        nc.vector.tensor_copy(lt[:], pl)
                        nc.sync.dma_start(out=LS[st], in_=lt[:])
                        scr = sm_pool.tile([128, 32], F32, tag="scr", name=f"scr_{st}")
                        dcol = sm_pool.tile([128, 1], F32, tag="dcol", name=f"dcol_{st}")
                        # (tensor_tensor_reduce hangs TRN2 here; use mul+reduce)
                        nc.vector.tensor_mul(scr[:], pl, ohd_t[:, st * 32:(st + 1) * 32])
                        nc.vector.tensor_reduce(dcol[:], scr[:], mybir.AxisListType.X, ALU.add)
                        nc.scalar.activation(wcol[:, st:st + 1], dcol[:], AF.Sigmoid)

                # --- experts ---
                for e in range(EPC):
                    # GEMM1/3: gT/uT [F-tile, Cp] accumulated over H;
                    # weights stationary (one 128x128 block per matmul).
                    ht_t = []
                    for fp_i in range(FP):
                        t1 = wt_pool.tile([128, 2048], F32R, tag="t1", name=f"t1_{e}_{fp_i}")
                        nc.sync.dma_start(out=t1[:], in_=wb1[e, fp_i])
                        t3 = wt_pool.tile([128, 2048], F32R, tag="t3", name=f"t3_{e}_{fp_i}")
                        nc.sync.dma_start(out=t3[:], in_=wb3[e, fp_i])
                        for j in range(2):
                            pg = psB.tile([128, Cp], F32, tag="gu", name=f"pg_{e}_{fp_i}_{j}")
                            pu = psB.tile([128, Cp], F32, tag="gu", name=f"pu_{e}_{fp_i}_{j}")
                            for hk in range(HK):
                                nc.tensor.matmul(
                                    pg[:], t1[:, (j * HK + hk) * 128:(j * HK + hk + 1) * 128],
                                    xg_t[e][:, hk * Cp:(hk + 1) * Cp],
                                    start=(hk == 0), stop=(hk == HK - 1),
                                )
                            for hk in range(HK):
                                nc.tensor.matmul(
                                    pu[:], t3[:, (j * HK + hk) * 128:(j * HK + hk + 1) * 128],
                                    xg_t[e][:, hk * Cp:(hk + 1) * Cp],
                                    start=(hk == 0), stop=(hk == HK - 1),
                                )
                            # silu(g)*u = g*sigmoid(g)*u ; DVE may read only one
                            # PSUM input per op, so evacuate g via ScalarE.
                            sg = sm_pool.tile([128, Cp], F32, tag="sg", name=f"sg_{e}_{fp_i}_{j}")
                            nc.scalar.activation(sg[:], pg[:], AF.Sigmoid)
                            gc = sm_pool.tile([128, Cp], F32, tag="gc", name=f"gc_{e}_{fp_i}_{j}")
                            nc.scalar.copy(gc[:], pg[:])
                            gu = sm_pool.tile([128, Cp], F32, tag="gu2", name=f"gu_{e}_{fp_i}_{j}")
                            nc.vector.tensor_mul(gu[:], gc[:], pu[:])
                            ht = ht_pool.tile([128, Cp], F32R, tag="ht", name=f"ht_{e}_{fp_i}_{j}")
                            nc.vector.tensor_mul(ht[:], gu[:], sg[:])
                            ht_t.append(ht)

                    # GEMM2: y [slots, H] accumulated over F; w2 moving.
                    for cpt_base in range(0, CPT, 2):
                        cpts = range(cpt_base, min(cpt_base + 2, CPT))
                        pys = {}
                        for cpt in cpts:
                            for hc in range(2):
                                pys[(cpt, hc)] = psA.tile([128, 512], F32, tag="y", name=f"py_{e}_{cpt}_{hc}")
                        for fp_i in range(FP):
                            t2 = wt_pool.tile([128, 2048], F32R, tag="t2", name=f"t2_{e}_{fp_i}")
                            nc.sync.dma_start(out=t2[:], in_=wb2[e, fp_i])
                            for j in range(2):
                                fk = fp_i * 2 + j
                                for cpt in cpts:
                                    for hc in range(2):
                                        nc.tensor.matmul(
                                            pys[(cpt, hc)][:],
                                            ht_t[fk][:, cpt * 128:(cpt + 1) * 128],
                                            t2[:, j * 1024 + hc * 512:j * 1024 + (hc + 1) * 512],
                                            start=(fk == 0), stop=(fk == FT - 1),
                                        )
                        for cpt in cpts:
                            st = e * CPT + cpt
                            for hc in range(2):
                                yw = yw_pool.tile([128, 512], F32, tag="yw", name=f"yw_{e}_{cpt}_{hc}")
                                nc.vector.tensor_scalar_mul(
                                    yw[:], pys[(cpt, hc)][:], wcol[:, st:st + 1])
                                nc.sync.dma_start(out=Y[e, cpt, hc], in_=yw[:])

            if repeat == 1:
                body()
            else:
                with tc.For_i(0, repeat, 1):
                    body()

    nc.compile()
    return nc


# ----------------------------------------------------------------------------
# Host-side dispatch / shard prep
# ----------------------------------------------------------------------------

def routing(selected_experts):
    """Reference-identical routing: stable sort by expert, rank in group."""
    flat_e = np.asarray(selected_experts, np.int64).reshape(-1)
    order = np.argsort(flat_e, kind="stable")
    sorted_e = flat_e[order]
    rank_sorted = np.arange(T * K) - np.searchsorted(sorted_e, sorted_e, side="left")
    rank_flat = np.empty(T * K, np.int64)
    rank_flat[order] = rank_sorted
    loads = np.bincount(flat_e, minlength=E)
    return flat_e, rank_flat, loads


def prepare(hidden_states, selected_experts, gate_w, w1, w2, w3):
    x = np.ascontiguousarray(np.asarray(hidden_states, np.float32).reshape(T, H))
    se = np.asarray(selected_experts)
    flat_e, rank_flat, loads = routing(se)

    Cp = int(min(CREF, max(256, 128 * math.ceil(max(int(loads.max()), 1) / 128))))
    valid = rank_flat < Cp        # == reference's rank < CREF whenever it matters
    CPT = Cp // 128
    NST = EPC * CPT

    # Gather tokens into global slot buffer [E*Cp, H]; pad slots stay zero.
    tok_flat = np.arange(T * K) // K
    slot_flat = flat_e * Cp + rank_flat
    XG = np.zeros((E * Cp, H), np.float32)
    XG[slot_flat[valid]] = x[tok_flat[valid]]

    # Per-slot +-1 selector over E for d = l_own - l_other  (zero row for
    # pad slots and for duplicate own==other -> d=0 -> w=0.5, as reference).
    OHD = np.zeros((E * Cp, E), np.float32)
    t_v = tok_flat[valid]
    k_v = (np.arange(T * K) % K)[valid]
    own = flat_e[valid]
    other = se[t_v, 1 - k_v].astype(np.int64)
    s_v = slot_flat[valid]
    np.add.at(OHD, (s_v, own), 1.0)
    np.add.at(OHD, (s_v, other), -1.0)

    w1r = np.asarray(w1, np.float32)
    w2r = np.asarray(w2, np.float32)
    w3r = np.asarray(w3, np.float32)
    gw = np.asarray(gate_w, np.float32)

    # Blocked transposed weight layouts (see build_nc for the slice math).
    #   wb1/wb3 [e,fp,p,(j,hk,f)]:  [E,FP,128,2048] with value w[(e, (fp*2+j)*128+f, hk*128+p)]
    wb1h = np.ascontiguousarray(
        w1r.reshape(E, FP, 2, 128, HK, 128).transpose(0, 1, 5, 2, 4, 3)
    ).reshape(E, FP, 128, 2048)
    wb3h = np.ascontiguousarray(
        w3r.reshape(E, FP, 2, 128, HK, 128).transpose(0, 1, 5, 2, 4, 3)
    ).reshape(E, FP, 128, 2048)
    #   wb2 [e,fkp,p,(j,h)]: value w2[e, h, (fkp*2+j)*128+p]
    wb2h = np.ascontiguousarray(
        w2r.reshape(E, H, FP, 2, 128).transpose(0, 2, 4, 3, 1)
    ).reshape(E, FP, 128, 2048)

    #   xg [e,p,(hk,c)]: value XG[e*Cp+c, hk*128+p]
    xgh = np.ascontiguousarray(
        XG.reshape(E, Cp, HK, 128).transpose(0, 3, 2, 1)
    ).reshape(E, 128, HK * Cp)

    gwh = np.ascontiguousarray(
        gw.reshape(E, HK, 128).transpose(2, 1, 0)
    ).reshape(128, HK * E)

    # per-core ohd: [128, NST*32] with value OHD[core_base + st*128 + p, e32]
    ohd_cores = []
    OHDc = OHD.reshape(NC, NST, 128, E)
    for c in range(NC):
        ohd_cores.append(np.ascontiguousarray(
            OHDc[c].transpose(1, 0, 2)).reshape(128, NST * E))

    in_maps = []
    for c in range(NC):
        sl = slice(c * EPC, (c + 1) * EPC)
        in_maps.append({
            "wb1": wb1h[sl], "wb3": wb3h[sl], "wb2": wb2h[sl],
            "xg": xgh[sl], "gwt": gwh, "ohd": ohd_cores[c],
        })

    meta = dict(Cp=Cp, CPT=CPT, NST=NST, flat_e=flat_e, rank_flat=rank_flat,
                valid=valid, x=x, gw=gw)
    return in_maps, meta


def assemble(results, meta):
    Cp = meta["Cp"]
    flat_e, rank_flat, valid = meta["flat_e"], meta["rank_flat"], meta["valid"]

    # [NC,EPC,CPT,2,128,512] -> global slot rows [E*Cp, H]
    Yall = np.stack([r["Y"] for r in results])
    Yflat = np.ascontiguousarray(
        Yall.transpose(0, 1, 2, 4, 3, 5)).reshape(E * Cp, H)
    LSall = np.stack([r["LS"] for r in results]).reshape(E * Cp, E)

    slot_flat = flat_e * Cp + rank_flat
    out = np.zeros((T, H), np.float32)
    rl = np.zeros((T, E), np.float32)
    rl_set = np.zeros(T, bool)
    for k in range(K):
        idx = np.arange(T) * K + k
        v = valid[idx]
        spos = np.where(v, slot_flat[idx], 0)
        out += Yflat[spos] * v[:, None]
        take = v & ~rl_set
        rl[take] = LSall[spos[take]]
        rl_set |= v
    if not rl_set.all():   # token dropped from all slots (essentially impossible)
        miss = ~rl_set
        rl[miss] = meta["x"][miss] @ meta["gw"].T
    return out.reshape(B, S, H), rl


# ----------------------------------------------------------------------------
# Entry point
# ----------------------------------------------------------------------------

_NC_CACHE = {}


def kernel(hidden_states, selected_experts, gate_w, w1, w2, w3):
    in_maps, meta = prepare(hidden_states, selected_experts, gate_w, w1, w2, w3)
    key = (meta["Cp"], 1)
    if key not in _NC_CACHE:
        _NC_CACHE[key] = build_nc(meta["Cp"], repeat=1)
    nc = _NC_CACHE[key]
    res = run_bass_kernel_spmd(nc, in_maps, core_ids=list(range(NC)))
    return assemble(res.results, meta)
